# revision 18
# baseline (speedup 1.0000x reference)
"""Mixture-of-logistics NLL loss (reduction=mean) on 8 Trainium2 NeuronCores.

Math (per row, K=16 mixture components):
    log_prob = ln(num) - ln(den)
    den = sum_k e^{w_k}
    num = sum_k e^{w_k} * pdf_k,  pdf = (1 - th^2) * rp / 4,
    th = tanh(0.5 * (t - loc) * rp),  rp = 1/s = exp(-ln(s))
Kernel accumulates stash_num = sum_k (th^2 - 1) * rp * e^w = -4*num per row;
the final ACT Ln uses scale=-0.25 so ln(-0.25*stash_num) = ln(num).
Output = per-partition partial sums [128, 2] = (sum ln(num), sum ln(den));
host combines mean = (sum col0 - sum col1)/N.

Sharding: pure data parallel over rows (batch*seq) across 8 cores.

Engine budget (per core, r=2048 rows/partition, K=16; full elementwise pass
over 32768 elems/partition: DVE 1x/2x/4x = 34.1/17.1/8.5us, ACT = 27.3us):
  ACT: Ln(s) + Exp(-u) + Exp(w) + Tanh + 3/13 Squares + final Lns ~= 137us
  DVE: sub(2x) + v-mul + q-mul + term path + two tree16 sums      ~= 137us
  DMA: 51.4MB HBM reads (~440GB/s/core measured) + 8.4MB SBUF t16 ~= 130us

ACT table sets: only Ln needs natural_log_exp_and_others and only Tanh needs
exp_and_others; Exp and Square are in BOTH. Tiles are processed in clumps:
phase A (Ln/Exp/Exp) of clump i is emitted before phase B (Tanh/Square) of
clump i-1, so the table set switches only 2x per clump (~9 loads total).

Tricks vs the previous version:
 - t16 = broadcast(t) materialized by SBUF->SBUF SWDGE DMA (idle queue time)
   so the (t - loc) subtract runs in 2x mode instead of 1x-broadcast.
 - th^2 on DVE uses neg(th) via tensor_scalar then th*(-th) (distinct
   operands keep 2x; same-operand th*th would drop to 1x), with the sign
   absorbed into the (th^2 - 1) tensor_scalar that follows.
 - q = rp*e^w overwrites rp in place; term = (th^2-1)*q overwrites the
   square tile; loc tile chains loc->diff->v->th.
 - bf16 everywhere on-chip (2x/4x DVE modes); SWDGE DMAs cast f32->bf16 in
   flight. Validated rel err ~3e-4 (gate 2e-2).
"""

import numpy as np

import concourse.bacc as bacc
import concourse.mybir as mybir
import concourse.tile as tile
from concourse.tile_rust import add_dep_helper
from concourse.bass_utils import run_bass_kernel_spmd

B, T, K = 16, 131072, 16
N = B * T                 # 2097152 rows total
NCORES = 8
NLOC = N // NCORES        # 262144 rows per core
P = 128                   # SBUF partitions

F32 = mybir.dt.float32
BF16 = mybir.dt.bfloat16
AF = mybir.ActivationFunctionType
OP = mybir.AluOpType


def build_kernel(nloc=NLOC, chunks=None, clump_sizes=None, act_sq=None):
    """Build the per-core Bass module.

    chunks: list of tuples of per-tile row counts (rows per partition).
    clump_sizes: how many chunks form one ACT table-set clump.
    act_sq: set of flat tile indices whose th^2 runs on ACT Square.
    """
    p = P
    r = nloc // p             # rows per partition (2048)
    if chunks is None:
        chunks = [(128,)] * 16
    if clump_sizes is None:
        clump_sizes = [4, 4, 4, 4]
    assert sum(sum(ch) for ch in chunks) == r and nloc % p == 0
    assert sum(clump_sizes) == len(chunks)
    cmax = max(max(ch) for ch in chunks)
    if act_sq is None:
        # ~9/16 of tiles square on ACT (balances ACT ~= DVE busy time)
        act_sq = {1, 3, 5, 6, 8, 10, 12, 14, 15}

    nc = bacc.Bacc("TRN2", target_bir_lowering=False, debug=False)
    w_d = nc.dram_tensor("w", [nloc, K], F32, kind="ExternalInput")
    loc_d = nc.dram_tensor("loc", [nloc, K], F32, kind="ExternalInput")
    scale_d = nc.dram_tensor("scale", [nloc, K], F32, kind="ExternalInput")
    t_d = nc.dram_tensor("t", [nloc], F32, kind="ExternalInput")
    out_d = nc.dram_tensor("out", [p, 2], F32, kind="ExternalOutput")

    wv = w_d.ap().rearrange("(p r) k -> p r k", p=p)
    lv = loc_d.ap().rearrange("(p r) k -> p r k", p=p)
    sv = scale_d.ap().rearrange("(p r) k -> p r k", p=p)
    tv = t_d.ap().rearrange("(p r) -> p r", p=p)

    acts = []  # every ACT instruction, in required execution order

    def act(*args, **kwargs):
        ins = nc.scalar.activation(*args, **kwargs)
        acts.append(ins)
        return ins

    with tile.TileContext(nc) as tc:
        with (
            tc.tile_pool(name="persist", bufs=1) as pp,
            tc.tile_pool(name="psc", bufs=8) as psc,
            tc.tile_pool(name="pwld", bufs=8) as pwld,
            tc.tile_pool(name="plc", bufs=8) as plc,
            tc.tile_pool(name="prp", bufs=8) as prp,
            tc.tile_pool(name="pc1", bufs=4) as pc1,
            nc.allow_low_precision("bf16 partial sums validated: ~3e-4 rel"),
        ):
            t_all = pp.tile([p, r], BF16)         # targets (bf16 master)
            stash2 = pp.tile([p, 2, r], F32)      # per-row (den | 4*num) sums
            out_sb = pp.tile([p, 2], F32)

            def tree2x16(q2, dst_slice, c):
                """Sum q2 [p, c, 2, 16] bf16 over last axis -> dst [p, c, 2] f32.

                One tree for both row-sums (den in [:,:,0,:], num in [:,:,1,:]):
                half the instruction count of two separate tree16s. Levels are
                written in place into q2's upper lanes (no temp tiles)."""
                nc.vector.tensor_add(out=q2[:, :, :, 8:16], in0=q2[:, :, :, 0:8],
                                     in1=q2[:, :, :, 8:16])
                nc.vector.tensor_add(out=q2[:, :, :, 4:8], in0=q2[:, :, :, 8:12],
                                     in1=q2[:, :, :, 12:16])
                nc.vector.tensor_add(out=q2[:, :, :, 2:4], in0=q2[:, :, :, 4:6],
                                     in1=q2[:, :, :, 6:8])
                nc.vector.tensor_add(out=dst_slice, in0=q2[:, :, :, 2],
                                     in1=q2[:, :, :, 3])

            

            off = 0
            starts = []
            for ch in chunks:
                starts.append(off)
                off += sum(ch)

            tix = [0]

            def emit_A(ci, ch):
                # ---- phase A of chunk: DMAs, Ln(s), Exp(-u), Exp(w), sub, v ----
                tinfo = []
                o = starts[ci]
                csl = slice(o, o + sum(ch))
                nc.gpsimd.dma_start(out=t_all[:, csl], in_=tv[:, csl])
                for c in ch:
                    sl = slice(o, o + c)
                    o += c
                    sc_t = psc.tile([p, cmax, K], BF16, tag="sc", name="sc")[:, :c, :]
                    # q2 holds e^w in [:,0,:,:] and later term in [:,1,:,:]
                    # (block layout keeps the w DMA destination contiguous)
                    q2 = pwld.tile([p, 2, cmax, K], BF16, tag="q2", name="q2t")[:, :, :c, :]
                    loc_t = plc.tile([p, cmax, K], BF16, tag="loc", name="loct")[:, :c, :]
                    # SWDGE DMAs cast f32->bf16 in flight
                    nc.gpsimd.dma_start(out=sc_t, in_=sv[:, sl, :])
                    nc.gpsimd.dma_start(out=q2[:, 0, :, :], in_=wv[:, sl, :])
                    nc.gpsimd.dma_start(out=loc_t, in_=lv[:, sl, :])
                    tinfo.append((sl, c, sc_t, q2, loc_t))

                # Batch all Lns, then all Exps: walrus gives Ln and Exp
                # different home tables, so interleaving would reload per tile.
                for sl, c, sc_t, q2, loc_t in tinfo:
                    act(out=sc_t, in_=sc_t, func=AF.Ln)                # u
                binfo = []
                for sl, c, sc_t, q2, loc_t in tinfo:
                    rp_t = prp.tile([p, cmax, K], BF16, tag="rp", name="rpt")[:, :c, :]
                    act(out=rp_t, in_=sc_t, func=AF.Exp, scale=-1.0)   # 1/s
                    act(out=q2[:, 0, :, :], in_=q2[:, 0, :, :], func=AF.Exp)  # e^w
                    # diff = t - loc (broadcast over K: 1x mode), v = diff*rp
                    tb = t_all[:, sl].unsqueeze(2).broadcast_to([p, c, K])
                    nc.vector.tensor_sub(out=loc_t, in0=tb, in1=loc_t)
                    nc.vector.tensor_mul(out=loc_t, in0=loc_t, in1=rp_t)
                    binfo.append((sl, c, loc_t, rp_t, q2))
                return binfo

            def emit_B(binfo):
                # ---- phase B: tanh, (1-th^2), q, term, combined tree ----
                for sl, c, v_t, rp_t, q2 in binfo:
                    act(out=v_t, in_=v_t, func=AF.Tanh, scale=0.5)     # th
                for sl, c, v_t, rp_t, q2 in binfo:
                    c1 = pc1.tile([p, cmax, K], BF16, tag="c1", name="c1t")[:, :c, :]
                    if tix[0] in act_sq:
                        # Square is in every table set: placement is free
                        act(out=c1, in_=v_t, func=AF.Square)           # th^2
                        nc.vector.tensor_scalar(
                            out=c1, in0=c1, scalar1=-1.0, scalar2=1.0,
                            op0=OP.mult, op1=OP.add,
                        )                                              # 1-th^2
                    else:
                        # neg-then-mul keeps 2x (same-operand th*th is 1x)
                        nc.vector.tensor_scalar(
                            out=c1, in0=v_t, scalar1=-1.0, scalar2=None,
                            op0=OP.mult,
                        )                                              # -th
                        nc.vector.tensor_mul(out=c1, in0=c1, in1=v_t)  # -th^2
                        nc.vector.tensor_scalar(
                            out=c1, in0=c1, scalar1=1.0, scalar2=1.0,
                            op0=OP.mult, op1=OP.add,
                        )                                              # 1-th^2
                    tix[0] += 1
                    ew = q2[:, 0, :, :]
                    nc.vector.tensor_mul(out=rp_t, in0=rp_t, in1=ew)   # q=rp*e^w
                    nc.vector.tensor_mul(out=q2[:, 1, :, :], in0=c1, in1=rp_t)
                    tree2x16(q2, stash2[:, :, sl], c)
                return binfo

            # Clumped software pipeline: A of clump i, then B of clump i-1.
            clumps = []
            ci = 0
            for cs in clump_sizes:
                clumps.append(list(range(ci, ci + cs)))
                ci += cs

            pending = None
            for cl in clumps:
                binfo = []
                for ci in cl:
                    binfo.extend(emit_A(ci, chunks[ci]))
                if pending is not None:
                    emit_B(pending)
                pending = binfo
            emit_B(pending)

            # ---- final: per-row logs + per-partition accumulation ----
            # stash2[:,:,1] = sum_k (1-th^2)*q = 4*num, so ln(num) uses
            # scale=0.25; stash2[:,:,0] = den.
            act(out=stash2[:, 1, :], in_=stash2[:, 1, :], func=AF.Ln,
                scale=0.25, accum_out=out_sb[:, 0:1])
            act(out=stash2[:, 0, :], in_=stash2[:, 0, :], func=AF.Ln,
                accum_out=out_sb[:, 1:2])
            nc.gpsimd.dma_start(out=out_d.ap(), in_=out_sb)

            # Pin ACT execution order (same engine -> scheduler-only edges)
            # so table-set switches stay at clump granularity.
            for prev, nxt in zip(acts, acts[1:]):
                add_dep_helper(nxt.ins, prev.ins, False, "act-table-order")

    nc.compile()
    return nc


def _combine(outs, n_rows):
    total = 0.0
    for o in outs:
        total += float(o[:, 0].sum(dtype=np.float64))
        total -= float(o[:, 1].sum(dtype=np.float64))
    return np.float32(total / n_rows)


def make_in_maps(weight, loc, scale, targets):
    w = np.ascontiguousarray(weight.reshape(N, K), dtype=np.float32)
    l = np.ascontiguousarray(loc.reshape(N, K), dtype=np.float32)
    s = np.ascontiguousarray(scale.reshape(N, K), dtype=np.float32)
    t = np.ascontiguousarray(targets.reshape(N), dtype=np.float32)
    in_maps = []
    for ci in range(NCORES):
        rs = slice(ci * NLOC, (ci + 1) * NLOC)
        in_maps.append({
            "w": np.ascontiguousarray(w[rs]),
            "loc": np.ascontiguousarray(l[rs]),
            "scale": np.ascontiguousarray(s[rs]),
            "t": np.ascontiguousarray(t[rs]),
        })
    return in_maps


def run(in_maps, **kwargs):
    nc = build_kernel()
    return run_bass_kernel_spmd(nc, in_maps, core_ids=list(range(NCORES)), **kwargs)


def kernel(weight, loc, scale, targets):
    in_maps = make_in_maps(weight, loc, scale, targets)
    last = None
    for _ in range(3):  # rare transient NRT device errors: retry
        try:
            res = run(in_maps)
            return _combine([r["out"] for r in res.results], N)
        except Exception as e:  # noqa: BLE001
            last = e
    raise last


if __name__ == "__main__":
    nc = build_kernel()
    print("kernel built OK")


# revision 19
# speedup vs baseline: 1.0147x; 1.0147x over previous
"""Mixture-of-logistics NLL loss (reduction=mean) on 8 Trainium2 NeuronCores.

Math (per row, K=16 mixture components):
    log_prob = ln(num) - ln(den)
    den = sum_k e^{w_k}
    num = sum_k e^{w_k} * pdf_k,  pdf = (1 - th^2) * rp / 4,
    th = tanh(0.5 * (t - loc) * rp),  rp = 1/s = exp(-ln(s))
Kernel accumulates stash_num = sum_k (th^2 - 1) * rp * e^w = -4*num per row;
the final ACT Ln uses scale=-0.25 so ln(-0.25*stash_num) = ln(num).
Output = per-partition partial sums [128, 2] = (sum ln(num), sum ln(den));
host combines mean = (sum col0 - sum col1)/N.

Sharding: pure data parallel over rows (batch*seq) across 8 cores.

Engine budget (per core, r=2048 rows/partition, K=16; full elementwise pass
over 32768 elems/partition: DVE 1x/2x/4x = 34.1/17.1/8.5us, ACT = 27.3us):
  ACT: Ln(s) + Exp(-u) + Exp(w) + Tanh + 3/13 Squares + final Lns ~= 137us
  DVE: sub(2x) + v-mul + q-mul + term path + two tree16 sums      ~= 137us
  DMA: 51.4MB HBM reads (~440GB/s/core measured) + 8.4MB SBUF t16 ~= 130us

ACT table sets: only Ln needs natural_log_exp_and_others and only Tanh needs
exp_and_others; Exp and Square are in BOTH. Tiles are processed in clumps:
phase A (Ln/Exp/Exp) of clump i is emitted before phase B (Tanh/Square) of
clump i-1, so the table set switches only 2x per clump (~9 loads total).

Tricks vs the previous version:
 - t16 = broadcast(t) materialized by SBUF->SBUF SWDGE DMA (idle queue time)
   so the (t - loc) subtract runs in 2x mode instead of 1x-broadcast.
 - th^2 on DVE uses neg(th) via tensor_scalar then th*(-th) (distinct
   operands keep 2x; same-operand th*th would drop to 1x), with the sign
   absorbed into the (th^2 - 1) tensor_scalar that follows.
 - q = rp*e^w overwrites rp in place; term = (th^2-1)*q overwrites the
   square tile; loc tile chains loc->diff->v->th.
 - bf16 everywhere on-chip (2x/4x DVE modes); SWDGE DMAs cast f32->bf16 in
   flight. Validated rel err ~3e-4 (gate 2e-2).
"""

import numpy as np

import concourse.bacc as bacc
import concourse.mybir as mybir
import concourse.tile as tile
from concourse.tile_rust import add_dep_helper
from concourse.bass_utils import run_bass_kernel_spmd

B, T, K = 16, 131072, 16
N = B * T                 # 2097152 rows total
NCORES = 8
NLOC = N // NCORES        # 262144 rows per core
P = 128                   # SBUF partitions

F32 = mybir.dt.float32
BF16 = mybir.dt.bfloat16
AF = mybir.ActivationFunctionType
OP = mybir.AluOpType


def build_kernel(nloc=NLOC, chunks=None, clump_sizes=None, act_sq=None):
    """Build the per-core Bass module.

    chunks: list of tuples of per-tile row counts (rows per partition).
    clump_sizes: how many chunks form one ACT table-set clump.
    act_sq: set of flat tile indices whose th^2 runs on ACT Square.
    """
    p = P
    r = nloc // p             # rows per partition (2048)
    if chunks is None:
        chunks = [(128, 128, 128, 128)] * 4
    if clump_sizes is None:
        clump_sizes = [1, 1, 1, 1]
    assert sum(sum(ch) for ch in chunks) == r and nloc % p == 0
    assert sum(clump_sizes) == len(chunks)
    cmax = max(max(ch) for ch in chunks)
    if act_sq is None:
        # ~9/16 of tiles square on ACT (balances ACT ~= DVE busy time)
        act_sq = {1, 3, 5, 6, 8, 10, 12, 14, 15}

    nc = bacc.Bacc("TRN2", target_bir_lowering=False, debug=False)
    w_d = nc.dram_tensor("w", [nloc, K], F32, kind="ExternalInput")
    loc_d = nc.dram_tensor("loc", [nloc, K], F32, kind="ExternalInput")
    scale_d = nc.dram_tensor("scale", [nloc, K], F32, kind="ExternalInput")
    t_d = nc.dram_tensor("t", [nloc], F32, kind="ExternalInput")
    out_d = nc.dram_tensor("out", [p, 2], F32, kind="ExternalOutput")

    wv = w_d.ap().rearrange("(p r) k -> p r k", p=p)
    lv = loc_d.ap().rearrange("(p r) k -> p r k", p=p)
    sv = scale_d.ap().rearrange("(p r) k -> p r k", p=p)
    tv = t_d.ap().rearrange("(p r) -> p r", p=p)

    acts = []  # every ACT instruction, in required execution order

    def act(*args, **kwargs):
        ins = nc.scalar.activation(*args, **kwargs)
        acts.append(ins)
        return ins

    with tile.TileContext(nc) as tc:
        with (
            tc.tile_pool(name="persist", bufs=1) as pp,
            tc.tile_pool(name="psc", bufs=8) as psc,
            tc.tile_pool(name="pwld", bufs=8) as pwld,
            tc.tile_pool(name="plc", bufs=8) as plc,
            tc.tile_pool(name="prp", bufs=8) as prp,
            tc.tile_pool(name="pc1", bufs=4) as pc1,
            nc.allow_low_precision("bf16 partial sums validated: ~3e-4 rel"),
        ):
            t_all = pp.tile([p, r], BF16)         # targets (bf16 master)
            stash2 = pp.tile([p, 2, r], F32)      # per-row (den | 4*num) sums
            out_sb = pp.tile([p, 2], F32)

            def tree16(h, dst_slice):
                """Sum h [p, c, 16] bf16 over last axis -> dst [p, c] f32.
                Levels are written in place into h's upper lanes (no temps;
                destroys h)."""
                nc.vector.tensor_add(out=h[:, :, 8:16], in0=h[:, :, 0:8],
                                     in1=h[:, :, 8:16])
                nc.vector.tensor_add(out=h[:, :, 4:8], in0=h[:, :, 8:12],
                                     in1=h[:, :, 12:16])
                nc.vector.tensor_add(out=h[:, :, 2:4], in0=h[:, :, 4:6],
                                     in1=h[:, :, 6:8])
                nc.vector.tensor_add(out=dst_slice, in0=h[:, :, 2],
                                     in1=h[:, :, 3])


            off = 0
            starts = []
            for ch in chunks:
                starts.append(off)
                off += sum(ch)

            tix = [0]

            def emit_A(ci, ch):
                # ---- phase A of chunk: DMAs, Ln(s), Exp(-u), Exp(w), sub, v ----
                tinfo = []
                o = starts[ci]
                csl = slice(o, o + sum(ch))
                nc.gpsimd.dma_start(out=t_all[:, csl], in_=tv[:, csl])
                for c in ch:
                    sl = slice(o, o + c)
                    o += c
                    sc_t = psc.tile([p, cmax, K], BF16, tag="sc", name="sc")[:, :c, :]
                    # q2 holds e^w in [:,0,:,:] and later term in [:,1,:,:]
                    # (block layout keeps the w DMA destination contiguous)
                    q2 = pwld.tile([p, 2, cmax, K], BF16, tag="q2", name="q2t")[:, :, :c, :]
                    loc_t = plc.tile([p, cmax, K], BF16, tag="loc", name="loct")[:, :c, :]
                    # SWDGE DMAs cast f32->bf16 in flight
                    nc.gpsimd.dma_start(out=sc_t, in_=sv[:, sl, :])
                    nc.gpsimd.dma_start(out=q2[:, 0, :, :], in_=wv[:, sl, :])
                    nc.gpsimd.dma_start(out=loc_t, in_=lv[:, sl, :])
                    tinfo.append((sl, c, sc_t, q2, loc_t))

                # Batch all Lns, then all Exps: walrus gives Ln and Exp
                # different home tables, so interleaving would reload per tile.
                for sl, c, sc_t, q2, loc_t in tinfo:
                    act(out=sc_t, in_=sc_t, func=AF.Ln)                # u
                binfo = []
                for sl, c, sc_t, q2, loc_t in tinfo:
                    rp_t = prp.tile([p, cmax, K], BF16, tag="rp", name="rpt")[:, :c, :]
                    act(out=rp_t, in_=sc_t, func=AF.Exp, scale=-1.0)   # 1/s
                    act(out=q2[:, 0, :, :], in_=q2[:, 0, :, :], func=AF.Exp)  # e^w
                    # diff = t - loc (broadcast over K: 1x mode), v = diff*rp
                    tb = t_all[:, sl].unsqueeze(2).broadcast_to([p, c, K])
                    nc.vector.tensor_sub(out=loc_t, in0=tb, in1=loc_t)
                    nc.vector.tensor_mul(out=loc_t, in0=loc_t, in1=rp_t)
                    # q = rp*e^w must precede the den tree (which destroys
                    # ew's upper lanes in place)
                    nc.vector.tensor_mul(out=rp_t, in0=rp_t, in1=q2[:, 0, :, :])
                    tree16(q2[:, 0, :, :], stash2[:, 0, sl])           # den
                    binfo.append((sl, c, loc_t, rp_t, q2))
                return binfo

            def emit_B(binfo):
                # ---- phase B: tanh, (1-th^2), q, term, combined tree ----
                for sl, c, v_t, rp_t, q2 in binfo:
                    act(out=v_t, in_=v_t, func=AF.Tanh, scale=0.5)     # th
                for sl, c, v_t, rp_t, q2 in binfo:
                    c1 = pc1.tile([p, cmax, K], BF16, tag="c1", name="c1t")[:, :c, :]
                    if tix[0] in act_sq:
                        # Square is in every table set: placement is free
                        act(out=c1, in_=v_t, func=AF.Square)           # th^2
                        nc.vector.tensor_scalar(
                            out=c1, in0=c1, scalar1=-1.0, scalar2=1.0,
                            op0=OP.mult, op1=OP.add,
                        )                                              # 1-th^2
                    else:
                        # neg-then-mul keeps 2x (same-operand th*th is 1x)
                        nc.vector.tensor_scalar(
                            out=c1, in0=v_t, scalar1=-1.0, scalar2=None,
                            op0=OP.mult,
                        )                                              # -th
                        nc.vector.tensor_mul(out=c1, in0=c1, in1=v_t)  # -th^2
                        nc.vector.tensor_scalar(
                            out=c1, in0=c1, scalar1=1.0, scalar2=1.0,
                            op0=OP.mult, op1=OP.add,
                        )                                              # 1-th^2
                    tix[0] += 1
                    nc.vector.tensor_mul(out=q2[:, 1, :, :], in0=c1, in1=rp_t)
                    tree16(q2[:, 1, :, :], stash2[:, 1, sl])           # num
                return binfo

            # Clumped software pipeline: A of clump i, then B of clump i-1.
            clumps = []
            ci = 0
            for cs in clump_sizes:
                clumps.append(list(range(ci, ci + cs)))
                ci += cs

            pending = None
            for cl in clumps:
                binfo = []
                for ci in cl:
                    binfo.extend(emit_A(ci, chunks[ci]))
                if pending is not None:
                    emit_B(pending)
                pending = binfo
            emit_B(pending)

            # ---- final: per-row logs + per-partition accumulation ----
            # stash2[:,:,1] = sum_k (1-th^2)*q = 4*num, so ln(num) uses
            # scale=0.25; stash2[:,:,0] = den.
            act(out=stash2[:, 1, :], in_=stash2[:, 1, :], func=AF.Ln,
                scale=0.25, accum_out=out_sb[:, 0:1])
            act(out=stash2[:, 0, :], in_=stash2[:, 0, :], func=AF.Ln,
                accum_out=out_sb[:, 1:2])
            nc.gpsimd.dma_start(out=out_d.ap(), in_=out_sb)

            # Pin ACT execution order (same engine -> scheduler-only edges)
            # so table-set switches stay at clump granularity.
            for prev, nxt in zip(acts, acts[1:]):
                add_dep_helper(nxt.ins, prev.ins, False, "act-table-order")

    nc.compile()
    return nc


def _combine(outs, n_rows):
    total = 0.0
    for o in outs:
        total += float(o[:, 0].sum(dtype=np.float64))
        total -= float(o[:, 1].sum(dtype=np.float64))
    return np.float32(total / n_rows)


def make_in_maps(weight, loc, scale, targets):
    w = np.ascontiguousarray(weight.reshape(N, K), dtype=np.float32)
    l = np.ascontiguousarray(loc.reshape(N, K), dtype=np.float32)
    s = np.ascontiguousarray(scale.reshape(N, K), dtype=np.float32)
    t = np.ascontiguousarray(targets.reshape(N), dtype=np.float32)
    in_maps = []
    for ci in range(NCORES):
        rs = slice(ci * NLOC, (ci + 1) * NLOC)
        in_maps.append({
            "w": np.ascontiguousarray(w[rs]),
            "loc": np.ascontiguousarray(l[rs]),
            "scale": np.ascontiguousarray(s[rs]),
            "t": np.ascontiguousarray(t[rs]),
        })
    return in_maps


def run(in_maps, **kwargs):
    nc = build_kernel()
    return run_bass_kernel_spmd(nc, in_maps, core_ids=list(range(NCORES)), **kwargs)


def kernel(weight, loc, scale, targets):
    in_maps = make_in_maps(weight, loc, scale, targets)
    last = None
    for _ in range(3):  # rare transient NRT device errors: retry
        try:
            res = run(in_maps)
            return _combine([r["out"] for r in res.results], N)
        except Exception as e:  # noqa: BLE001
            last = e
    raise last


if __name__ == "__main__":
    nc = build_kernel()
    print("kernel built OK")


# revision 20
# speedup vs baseline: 1.0423x; 1.0271x over previous
"""Mixture-of-logistics NLL loss (reduction=mean) on 8 Trainium2 NeuronCores.

Math (per row, K=16 mixture components):
    log_prob = ln(num) - ln(den)
    den = sum_k e^{w_k}
    num = sum_k e^{w_k} * pdf_k,  pdf = (1 - th^2) * rp / 4,
    th = tanh(0.5 * (t - loc) * rp),  rp = 1/s = exp(-ln(s))
Kernel accumulates stash_num = sum_k (th^2 - 1) * rp * e^w = -4*num per row;
the final ACT Ln uses scale=-0.25 so ln(-0.25*stash_num) = ln(num).
Output = per-partition partial sums [128, 2] = (sum ln(num), sum ln(den));
host combines mean = (sum col0 - sum col1)/N.

Sharding: pure data parallel over rows (batch*seq) across 8 cores.

Engine budget (per core, r=2048 rows/partition, K=16; full elementwise pass
over 32768 elems/partition: DVE 1x/2x/4x = 34.1/17.1/8.5us, ACT = 27.3us):
  ACT: Ln(s) + Exp(-u) + Exp(w) + Tanh + 3/13 Squares + final Lns ~= 137us
  DVE: sub(2x) + v-mul + q-mul + term path + two tree16 sums      ~= 137us
  DMA: 51.4MB HBM reads (~440GB/s/core measured) + 8.4MB SBUF t16 ~= 130us

ACT table sets: only Ln needs natural_log_exp_and_others and only Tanh needs
exp_and_others; Exp and Square are in BOTH. Tiles are processed in clumps:
phase A (Ln/Exp/Exp) of clump i is emitted before phase B (Tanh/Square) of
clump i-1, so the table set switches only 2x per clump (~9 loads total).

Tricks vs the previous version:
 - t16 = broadcast(t) materialized by SBUF->SBUF SWDGE DMA (idle queue time)
   so the (t - loc) subtract runs in 2x mode instead of 1x-broadcast.
 - th^2 on DVE uses neg(th) via tensor_scalar then th*(-th) (distinct
   operands keep 2x; same-operand th*th would drop to 1x), with the sign
   absorbed into the (th^2 - 1) tensor_scalar that follows.
 - q = rp*e^w overwrites rp in place; term = (th^2-1)*q overwrites the
   square tile; loc tile chains loc->diff->v->th.
 - bf16 everywhere on-chip (2x/4x DVE modes); SWDGE DMAs cast f32->bf16 in
   flight. Validated rel err ~3e-4 (gate 2e-2).
"""

import numpy as np

import concourse.bacc as bacc
import concourse.mybir as mybir
import concourse.tile as tile
from concourse.tile_rust import add_dep_helper
from concourse.bass_utils import run_bass_kernel_spmd

B, T, K = 16, 131072, 16
N = B * T                 # 2097152 rows total
NCORES = 8
NLOC = N // NCORES        # 262144 rows per core
P = 128                   # SBUF partitions

F32 = mybir.dt.float32
BF16 = mybir.dt.bfloat16
AF = mybir.ActivationFunctionType
OP = mybir.AluOpType


def build_kernel(nloc=NLOC, chunks=None, clump_sizes=None, act_sq=None):
    """Build the per-core Bass module.

    chunks: list of tuples of per-tile row counts (rows per partition).
    clump_sizes: how many chunks form one ACT table-set clump.
    act_sq: set of flat tile indices whose th^2 runs on ACT Square.
    """
    p = P
    r = nloc // p             # rows per partition (2048)
    if chunks is None:
        # graduated sizes: small edge chunks shorten pipeline fill/drain
        chunks = [(32, 64, 96), (128, 160, 192), (192, 192, 192),
                  (192, 192, 160), (128, 96, 32)]
    if clump_sizes is None:
        clump_sizes = [1] * 5
    assert sum(sum(ch) for ch in chunks) == r and nloc % p == 0
    assert sum(clump_sizes) == len(chunks)
    cmax = max(max(ch) for ch in chunks)
    if act_sq is None:
        # tiles whose th^2 runs on ACT Square: ~56% of rows (the big middle
        # tiles), balancing ACT ~= DVE busy time
        act_sq = {5, 6, 7, 8, 9, 10}

    nc = bacc.Bacc("TRN2", target_bir_lowering=False, debug=False)
    w_d = nc.dram_tensor("w", [nloc, K], F32, kind="ExternalInput")
    loc_d = nc.dram_tensor("loc", [nloc, K], F32, kind="ExternalInput")
    scale_d = nc.dram_tensor("scale", [nloc, K], F32, kind="ExternalInput")
    t_d = nc.dram_tensor("t", [nloc], F32, kind="ExternalInput")
    out_d = nc.dram_tensor("out", [p, 2], F32, kind="ExternalOutput")

    wv = w_d.ap().rearrange("(p r) k -> p r k", p=p)
    lv = loc_d.ap().rearrange("(p r) k -> p r k", p=p)
    sv = scale_d.ap().rearrange("(p r) k -> p r k", p=p)
    tv = t_d.ap().rearrange("(p r) -> p r", p=p)

    acts = []  # every ACT instruction, in required execution order

    def act(*args, **kwargs):
        ins = nc.scalar.activation(*args, **kwargs)
        acts.append(ins)
        return ins

    with tile.TileContext(nc) as tc:
        with (
            tc.tile_pool(name="persist", bufs=1) as pp,
            tc.tile_pool(name="psc", bufs=6) as psc,
            tc.tile_pool(name="pwld", bufs=6) as pwld,
            tc.tile_pool(name="plc", bufs=7) as plc,
            tc.tile_pool(name="prp", bufs=7) as prp,
            tc.tile_pool(name="pc1", bufs=4) as pc1,
            nc.allow_low_precision("bf16 partial sums validated: ~3e-4 rel"),
        ):
            t_all = pp.tile([p, r], BF16)         # targets (bf16 master)
            stash2 = pp.tile([p, 2, r], F32)      # per-row (den | 4*num) sums
            out_sb = pp.tile([p, 2], F32)
            nc.gpsimd.dma_start(out=t_all, in_=tv)

            def tree16(h, dst_slice):
                """Sum h [p, c, 16] bf16 over last axis -> dst [p, c] f32.
                Levels are written in place into h's upper lanes (no temps;
                destroys h)."""
                nc.vector.tensor_add(out=h[:, :, 8:16], in0=h[:, :, 0:8],
                                     in1=h[:, :, 8:16])
                nc.vector.tensor_add(out=h[:, :, 4:8], in0=h[:, :, 8:12],
                                     in1=h[:, :, 12:16])
                nc.vector.tensor_add(out=h[:, :, 2:4], in0=h[:, :, 4:6],
                                     in1=h[:, :, 6:8])
                nc.vector.tensor_add(out=dst_slice, in0=h[:, :, 2],
                                     in1=h[:, :, 3])


            off = 0
            starts = []
            for ch in chunks:
                starts.append(off)
                off += sum(ch)

            tix = [0]

            def emit_A(ci, ch):
                # ---- phase A of chunk: DMAs, Ln(s), Exp(-u), Exp(w), sub, v ----
                tinfo = []
                o = starts[ci]
                for c in ch:
                    sl = slice(o, o + c)
                    o += c
                    sc_t = psc.tile([p, cmax, K], BF16, tag="sc", name="sc")[:, :c, :]
                    w_t = pwld.tile([p, cmax, K], BF16, tag="w", name="wt")[:, :c, :]
                    loc_t = plc.tile([p, cmax, K], BF16, tag="loc", name="loct")[:, :c, :]
                    # SWDGE DMAs cast f32->bf16 in flight
                    nc.gpsimd.dma_start(out=sc_t, in_=sv[:, sl, :])
                    nc.gpsimd.dma_start(out=w_t, in_=wv[:, sl, :])
                    nc.gpsimd.dma_start(out=loc_t, in_=lv[:, sl, :])
                    tinfo.append((sl, c, sc_t, w_t, loc_t))

                # Batch all Lns, then all Exps: walrus gives Ln and Exp
                # different home tables, so interleaving would reload per tile.
                for sl, c, sc_t, w_t, loc_t in tinfo:
                    act(out=sc_t, in_=sc_t, func=AF.Ln)                # u
                binfo = []
                for sl, c, sc_t, w_t, loc_t in tinfo:
                    rp_t = prp.tile([p, cmax, K], BF16, tag="rp", name="rpt")[:, :c, :]
                    act(out=rp_t, in_=sc_t, func=AF.Exp, scale=-1.0)   # 1/s
                    act(out=w_t, in_=w_t, func=AF.Exp)                 # e^w
                    # diff = t - loc (broadcast over K: 1x mode), v = diff*rp
                    tb = t_all[:, sl].unsqueeze(2).broadcast_to([p, c, K])
                    nc.vector.tensor_sub(out=loc_t, in0=tb, in1=loc_t)
                    nc.vector.tensor_mul(out=loc_t, in0=loc_t, in1=rp_t)
                    # q = rp*e^w must precede the den tree (which destroys
                    # ew's upper lanes in place)
                    nc.vector.tensor_mul(out=rp_t, in0=rp_t, in1=w_t)
                    tree16(w_t, stash2[:, 0, sl])                      # den
                    binfo.append((sl, c, loc_t, rp_t))
                return binfo

            def emit_B(binfo):
                # ---- phase B: tanh, (1-th^2), term, num tree ----
                for sl, c, v_t, rp_t in binfo:
                    act(out=v_t, in_=v_t, func=AF.Tanh, scale=0.5)     # th
                for sl, c, v_t, rp_t in binfo:
                    c1 = pc1.tile([p, cmax, K], BF16, tag="c1", name="c1t")[:, :c, :]
                    if tix[0] in act_sq:
                        # Square is in every table set: placement is free
                        act(out=c1, in_=v_t, func=AF.Square)           # th^2
                        nc.vector.tensor_scalar(
                            out=c1, in0=c1, scalar1=-1.0, scalar2=1.0,
                            op0=OP.mult, op1=OP.add,
                        )                                              # 1-th^2
                    else:
                        # neg-then-mul keeps 2x (same-operand th*th is 1x)
                        nc.vector.tensor_scalar(
                            out=c1, in0=v_t, scalar1=-1.0, scalar2=None,
                            op0=OP.mult,
                        )                                              # -th
                        nc.vector.tensor_mul(out=c1, in0=c1, in1=v_t)  # -th^2
                        nc.vector.tensor_scalar(
                            out=c1, in0=c1, scalar1=1.0, scalar2=1.0,
                            op0=OP.mult, op1=OP.add,
                        )                                              # 1-th^2
                    tix[0] += 1
                    nc.vector.tensor_mul(out=c1, in0=c1, in1=rp_t)     # term
                    tree16(c1, stash2[:, 1, sl])                       # num
                return binfo

            # Clumped software pipeline: A of clump i, then B of clump i-1.
            clumps = []
            ci = 0
            for cs in clump_sizes:
                clumps.append(list(range(ci, ci + cs)))
                ci += cs

            pending = None
            for cl in clumps:
                binfo = []
                for ci in cl:
                    binfo.extend(emit_A(ci, chunks[ci]))
                if pending is not None:
                    emit_B(pending)
                pending = binfo
            emit_B(pending)

            # ---- final: per-row logs + per-partition accumulation ----
            # stash2[:,:,1] = sum_k (1-th^2)*q = 4*num, so ln(num) uses
            # scale=0.25; stash2[:,:,0] = den.
            act(out=stash2[:, 1, :], in_=stash2[:, 1, :], func=AF.Ln,
                scale=0.25, accum_out=out_sb[:, 0:1])
            act(out=stash2[:, 0, :], in_=stash2[:, 0, :], func=AF.Ln,
                accum_out=out_sb[:, 1:2])
            nc.gpsimd.dma_start(out=out_d.ap(), in_=out_sb)

            # Pin ACT execution order (same engine -> scheduler-only edges)
            # so table-set switches stay at clump granularity.
            for prev, nxt in zip(acts, acts[1:]):
                add_dep_helper(nxt.ins, prev.ins, False, "act-table-order")

    nc.compile()
    return nc


def _combine(outs, n_rows):
    total = 0.0
    for o in outs:
        total += float(o[:, 0].sum(dtype=np.float64))
        total -= float(o[:, 1].sum(dtype=np.float64))
    return np.float32(total / n_rows)


def make_in_maps(weight, loc, scale, targets):
    w = np.ascontiguousarray(weight.reshape(N, K), dtype=np.float32)
    l = np.ascontiguousarray(loc.reshape(N, K), dtype=np.float32)
    s = np.ascontiguousarray(scale.reshape(N, K), dtype=np.float32)
    t = np.ascontiguousarray(targets.reshape(N), dtype=np.float32)
    in_maps = []
    for ci in range(NCORES):
        rs = slice(ci * NLOC, (ci + 1) * NLOC)
        in_maps.append({
            "w": np.ascontiguousarray(w[rs]),
            "loc": np.ascontiguousarray(l[rs]),
            "scale": np.ascontiguousarray(s[rs]),
            "t": np.ascontiguousarray(t[rs]),
        })
    return in_maps


def run(in_maps, **kwargs):
    nc = build_kernel()
    return run_bass_kernel_spmd(nc, in_maps, core_ids=list(range(NCORES)), **kwargs)


def kernel(weight, loc, scale, targets):
    in_maps = make_in_maps(weight, loc, scale, targets)
    last = None
    for _ in range(3):  # rare transient NRT device errors: retry
        try:
            res = run(in_maps)
            return _combine([r["out"] for r in res.results], N)
        except Exception as e:  # noqa: BLE001
            last = e
    raise last


if __name__ == "__main__":
    nc = build_kernel()
    print("kernel built OK")


# revision 21
# speedup vs baseline: 1.0576x; 1.0148x over previous
"""Mixture-of-logistics NLL loss (reduction=mean) on 8 Trainium2 NeuronCores.

Math (per row, K=16 mixture components):
    log_prob = ln(num) - ln(den)
    den = sum_k e^{w_k}
    num = sum_k e^{w_k} * pdf_k,  pdf = (1 - th^2) * rp / 4,
    th = tanh(0.5 * (t - loc) * rp),  rp = 1/s = exp(-ln(s))
Kernel accumulates stash_num = sum_k (th^2 - 1) * rp * e^w = -4*num per row;
the final ACT Ln uses scale=-0.25 so ln(-0.25*stash_num) = ln(num).
Output = per-partition partial sums [128, 2] = (sum ln(num), sum ln(den));
host combines mean = (sum col0 - sum col1)/N.

Sharding: pure data parallel over rows (batch*seq) across 8 cores.

Engine budget (per core, r=2048 rows/partition, K=16; full elementwise pass
over 32768 elems/partition: DVE 1x/2x/4x = 34.1/17.1/8.5us, ACT = 27.3us):
  ACT: Ln(s) + Exp(-u) + Exp(w) + Tanh + 3/13 Squares + final Lns ~= 137us
  DVE: sub(2x) + v-mul + q-mul + term path + two tree16 sums      ~= 137us
  DMA: 51.4MB HBM reads (~440GB/s/core measured) + 8.4MB SBUF t16 ~= 130us

ACT table sets: only Ln needs natural_log_exp_and_others and only Tanh needs
exp_and_others; Exp and Square are in BOTH. Tiles are processed in clumps:
phase A (Ln/Exp/Exp) of clump i is emitted before phase B (Tanh/Square) of
clump i-1, so the table set switches only 2x per clump (~9 loads total).

Tricks vs the previous version:
 - t16 = broadcast(t) materialized by SBUF->SBUF SWDGE DMA (idle queue time)
   so the (t - loc) subtract runs in 2x mode instead of 1x-broadcast.
 - th^2 on DVE uses neg(th) via tensor_scalar then th*(-th) (distinct
   operands keep 2x; same-operand th*th would drop to 1x), with the sign
   absorbed into the (th^2 - 1) tensor_scalar that follows.
 - q = rp*e^w overwrites rp in place; term = (th^2-1)*q overwrites the
   square tile; loc tile chains loc->diff->v->th.
 - bf16 everywhere on-chip (2x/4x DVE modes); SWDGE DMAs cast f32->bf16 in
   flight. Validated rel err ~3e-4 (gate 2e-2).
"""

import numpy as np

import concourse.bacc as bacc
import concourse.mybir as mybir
import concourse.tile as tile
from concourse.tile_rust import add_dep_helper
from concourse.bass_utils import run_bass_kernel_spmd

B, T, K = 16, 131072, 16
N = B * T                 # 2097152 rows total
NCORES = 8
NLOC = N // NCORES        # 262144 rows per core
P = 128                   # SBUF partitions

F32 = mybir.dt.float32
BF16 = mybir.dt.bfloat16
AF = mybir.ActivationFunctionType
OP = mybir.AluOpType


def build_kernel(nloc=NLOC, chunks=None, clump_sizes=None, act_sq=None):
    """Build the per-core Bass module.

    chunks: list of tuples of per-tile row counts (rows per partition).
    clump_sizes: how many chunks form one ACT table-set clump.
    act_sq: set of flat tile indices whose th^2 runs on ACT Square.
    """
    p = P
    r = nloc // p             # rows per partition (2048)
    if chunks is None:
        # graduated sizes: small edge chunks shorten pipeline fill/drain
        chunks = [(32, 64, 96), (128, 160, 192), (192, 192, 192),
                  (192, 192, 160), (128, 96, 32)]
    if clump_sizes is None:
        clump_sizes = [1] * 5
    assert sum(sum(ch) for ch in chunks) == r and nloc % p == 0
    assert sum(clump_sizes) == len(chunks)
    cmax = max(max(ch) for ch in chunks)
    if act_sq is None:
        # tiles whose th^2 runs on ACT Square: ~56% of rows (the big middle
        # tiles), balancing ACT ~= DVE busy time
        act_sq = {5, 6, 7, 8, 9, 10}

    nc = bacc.Bacc("TRN2", target_bir_lowering=False, debug=False)
    w_d = nc.dram_tensor("w", [nloc, K], F32, kind="ExternalInput")
    loc_d = nc.dram_tensor("loc", [nloc, K], F32, kind="ExternalInput")
    scale_d = nc.dram_tensor("scale", [nloc, K], F32, kind="ExternalInput")
    t_d = nc.dram_tensor("t", [nloc], F32, kind="ExternalInput")
    out_d = nc.dram_tensor("out", [p, 2], F32, kind="ExternalOutput")

    wv = w_d.ap().rearrange("(p r) k -> p r k", p=p)
    lv = loc_d.ap().rearrange("(p r) k -> p r k", p=p)
    sv = scale_d.ap().rearrange("(p r) k -> p r k", p=p)
    tv = t_d.ap().rearrange("(p r) -> p r", p=p)

    acts = []  # every ACT instruction, in required execution order

    def act(*args, **kwargs):
        ins = nc.scalar.activation(*args, **kwargs)
        acts.append(ins)
        return ins

    with tile.TileContext(nc) as tc:
        with (
            tc.tile_pool(name="persist", bufs=1) as pp,
            tc.tile_pool(name="psc", bufs=4) as psc,
            tc.tile_pool(name="pwld", bufs=4) as pwld,
            tc.tile_pool(name="plc", bufs=4) as plc,
            tc.tile_pool(name="prp", bufs=3) as prp,
            tc.tile_pool(name="pv", bufs=6) as pv,
            tc.tile_pool(name="pq", bufs=6) as pq,
            tc.tile_pool(name="pc1", bufs=3) as pc1,
            nc.allow_low_precision("bf16 partial sums validated: ~3e-4 rel"),
        ):
            t_all = pp.tile([p, r], BF16)         # targets (bf16 master)
            stash2 = pp.tile([p, 2, r], F32)      # per-row (den | 4*num) sums
            out_sb = pp.tile([p, 2], F32)
            nc.gpsimd.dma_start(out=t_all, in_=tv)

            def tree16(h, dst_slice):
                """Sum h [p, c, 16] bf16 over last axis -> dst [p, c] f32.
                Levels are written in place into h's upper lanes (no temps;
                destroys h)."""
                nc.vector.tensor_add(out=h[:, :, 8:16], in0=h[:, :, 0:8],
                                     in1=h[:, :, 8:16])
                nc.vector.tensor_add(out=h[:, :, 4:8], in0=h[:, :, 8:12],
                                     in1=h[:, :, 12:16])
                nc.vector.tensor_add(out=h[:, :, 2:4], in0=h[:, :, 4:6],
                                     in1=h[:, :, 6:8])
                nc.vector.tensor_add(out=dst_slice, in0=h[:, :, 2],
                                     in1=h[:, :, 3])


            off = 0
            starts = []
            for ch in chunks:
                starts.append(off)
                off += sum(ch)

            tix = [0]

            def emit_A(ci, ch):
                # ---- phase A of chunk: DMAs, Ln(s), Exp(-u), Exp(w), sub, v ----
                tinfo = []
                o = starts[ci]
                for c in ch:
                    sl = slice(o, o + c)
                    o += c
                    sc_t = psc.tile([p, cmax, K], BF16, tag="sc", name="sc")[:, :c, :]
                    w_t = pwld.tile([p, cmax, K], BF16, tag="w", name="wt")[:, :c, :]
                    loc_t = plc.tile([p, cmax, K], BF16, tag="loc", name="loct")[:, :c, :]
                    # SWDGE DMAs cast f32->bf16 in flight
                    nc.gpsimd.dma_start(out=sc_t, in_=sv[:, sl, :])
                    nc.gpsimd.dma_start(out=w_t, in_=wv[:, sl, :])
                    nc.gpsimd.dma_start(out=loc_t, in_=lv[:, sl, :])
                    tinfo.append((sl, c, sc_t, w_t, loc_t))

                # Batch all Lns, then all Exps: walrus gives Ln and Exp
                # different home tables, so interleaving would reload per tile.
                for sl, c, sc_t, w_t, loc_t in tinfo:
                    act(out=sc_t, in_=sc_t, func=AF.Ln)                # u
                binfo = []
                for sl, c, sc_t, w_t, loc_t in tinfo:
                    rp_t = prp.tile([p, cmax, K], BF16, tag="rp", name="rpt")[:, :c, :]
                    act(out=rp_t, in_=sc_t, func=AF.Exp, scale=-1.0)   # 1/s
                    act(out=w_t, in_=w_t, func=AF.Exp)                 # e^w
                    # diff = t - loc (broadcast over K: 1x mode) in place;
                    # v/q go to FRESH tiles so the DMA-fed input tiles (loc,
                    # w) recycle fast and the DMA never starves for buffers.
                    tb = t_all[:, sl].unsqueeze(2).broadcast_to([p, c, K])
                    nc.vector.tensor_sub(out=loc_t, in0=tb, in1=loc_t)
                    v_t = pv.tile([p, cmax, K], BF16, tag="v", name="vt")[:, :c, :]
                    nc.vector.tensor_mul(out=v_t, in0=loc_t, in1=rp_t)
                    q_t = pq.tile([p, cmax, K], BF16, tag="q", name="qt")[:, :c, :]
                    nc.vector.tensor_mul(out=q_t, in0=rp_t, in1=w_t)   # q=rp*e^w
                    tree16(w_t, stash2[:, 0, sl])                      # den
                    binfo.append((sl, c, v_t, q_t))
                return binfo

            def emit_B(binfo):
                # ---- phase B: tanh, (1-th^2), term, num tree ----
                for sl, c, v_t, q_t in binfo:
                    act(out=v_t, in_=v_t, func=AF.Tanh, scale=0.5)     # th
                for sl, c, v_t, q_t in binfo:
                    c1 = pc1.tile([p, cmax, K], BF16, tag="c1", name="c1t")[:, :c, :]
                    if tix[0] in act_sq:
                        # Square is in every table set: placement is free
                        act(out=c1, in_=v_t, func=AF.Square)           # th^2
                        nc.vector.tensor_scalar(
                            out=c1, in0=c1, scalar1=-1.0, scalar2=1.0,
                            op0=OP.mult, op1=OP.add,
                        )                                              # 1-th^2
                    else:
                        # neg-then-mul keeps 2x (same-operand th*th is 1x)
                        nc.vector.tensor_scalar(
                            out=c1, in0=v_t, scalar1=-1.0, scalar2=None,
                            op0=OP.mult,
                        )                                              # -th
                        nc.vector.tensor_mul(out=c1, in0=c1, in1=v_t)  # -th^2
                        nc.vector.tensor_scalar(
                            out=c1, in0=c1, scalar1=1.0, scalar2=1.0,
                            op0=OP.mult, op1=OP.add,
                        )                                              # 1-th^2
                    tix[0] += 1
                    nc.vector.tensor_mul(out=c1, in0=c1, in1=q_t)      # term
                    tree16(c1, stash2[:, 1, sl])                       # num
                return binfo

            # Clumped software pipeline: A of clump i, then B of clump i-1.
            clumps = []
            ci = 0
            for cs in clump_sizes:
                clumps.append(list(range(ci, ci + cs)))
                ci += cs

            pending = None
            for cl in clumps:
                binfo = []
                for ci in cl:
                    binfo.extend(emit_A(ci, chunks[ci]))
                if pending is not None:
                    emit_B(pending)
                pending = binfo
            emit_B(pending)

            # ---- final: per-row logs + per-partition accumulation ----
            # stash2[:,:,1] = sum_k (1-th^2)*q = 4*num, so ln(num) uses
            # scale=0.25; stash2[:,:,0] = den.
            act(out=stash2[:, 1, :], in_=stash2[:, 1, :], func=AF.Ln,
                scale=0.25, accum_out=out_sb[:, 0:1])
            act(out=stash2[:, 0, :], in_=stash2[:, 0, :], func=AF.Ln,
                accum_out=out_sb[:, 1:2])
            nc.gpsimd.dma_start(out=out_d.ap(), in_=out_sb)

            # Pin ACT execution order (same engine -> scheduler-only edges)
            # so table-set switches stay at clump granularity.
            for prev, nxt in zip(acts, acts[1:]):
                add_dep_helper(nxt.ins, prev.ins, False, "act-table-order")

    nc.compile()
    return nc


def _combine(outs, n_rows):
    total = 0.0
    for o in outs:
        total += float(o[:, 0].sum(dtype=np.float64))
        total -= float(o[:, 1].sum(dtype=np.float64))
    return np.float32(total / n_rows)


def make_in_maps(weight, loc, scale, targets):
    w = np.ascontiguousarray(weight.reshape(N, K), dtype=np.float32)
    l = np.ascontiguousarray(loc.reshape(N, K), dtype=np.float32)
    s = np.ascontiguousarray(scale.reshape(N, K), dtype=np.float32)
    t = np.ascontiguousarray(targets.reshape(N), dtype=np.float32)
    in_maps = []
    for ci in range(NCORES):
        rs = slice(ci * NLOC, (ci + 1) * NLOC)
        in_maps.append({
            "w": np.ascontiguousarray(w[rs]),
            "loc": np.ascontiguousarray(l[rs]),
            "scale": np.ascontiguousarray(s[rs]),
            "t": np.ascontiguousarray(t[rs]),
        })
    return in_maps


def run(in_maps, **kwargs):
    nc = build_kernel()
    return run_bass_kernel_spmd(nc, in_maps, core_ids=list(range(NCORES)), **kwargs)


def kernel(weight, loc, scale, targets):
    in_maps = make_in_maps(weight, loc, scale, targets)
    last = None
    for _ in range(3):  # rare transient NRT device errors: retry
        try:
            res = run(in_maps)
            return _combine([r["out"] for r in res.results], N)
        except Exception as e:  # noqa: BLE001
            last = e
    raise last


if __name__ == "__main__":
    nc = build_kernel()
    print("kernel built OK")
